# revision 67
# baseline (speedup 1.0000x reference)
"""Trainium2 Bass kernel for nn_ColumnEncoding (bidirectional masked LSTM
over 4096 split-delimited token segments).

Sharding: data-parallel over the 4096 columns -> 512 columns per core on 8
cores.  LSTM weights are replicated.  Each core runs an identical SPMD Bass
program on its shard; the host concatenates the 8 [512, 512] outputs.

The input-side gate pre-activations W_in . E[tok] + b depend only on the
token id, so the host precomputes a per-vocab gates table once per weight
set (one 39-GFLOP BLAS matmul, cached) and per call just gathers rows for
the 32768 tokens.  The device receives the input gates pre-computed in bf16
and injects each [128, 512] block into PSUM with a single identity matmul
(512 PE cycles instead of the 3x512 the three K-tile input matmuls cost),
then accumulates only the recurrent W_hh . h_{t-1} matmuls on top.  This
cuts TensorE time ~40% below the classic dataflow; the ScalarE activation
wall becomes the bound.

Per-core device pipeline:
  1. Input-gate blocks ([128, 8*512] bf16 per (step, direction)) stream in
     on two DMA queues in use order; recurrent weights and the column mask
     load on the Act queue.  The first matmul starts ~2.4us in.
  2. For each step t (8) and direction (fwd l=t / bwd l=7-t), gates^T
     [1024, 512cols] are built in four 2-bank per-gate PSUM units (i, g,
     f, o order) as identity-matmul injection of the precomputed input
     gates + W_hh^T @ h_{t-1} (2 K-tiles, skipped at t=0).
  3. ScalarE applies sigmoid/tanh per gate (bf16 outputs); VectorE does the
     cell update (c in fp32, products in bf16 for the 2x/4x DVE modes); h is
     written bf16 and fed back as the next matmul rhs.
  4. The ragged first column (segment length 7 instead of 8) is handled with
     tiny column-0 masked ops ([128, 2] slices, per-core mask data) and a
     host-side fix from the step-6 hidden (out6), keeping the program SPMD.
  5. Final hidden states stream out per hid-half during the last step; the
     host untangles [hid, col] -> [col, feat].
"""

import numpy as np
import ml_dtypes

VOCAB = 32000
EMBED = 300
HID = 256
N_COLS = 4096
SEG_LEN = 8
T = N_COLS * SEG_LEN
NCORES = 8
COLS = N_COLS // NCORES          # 512 columns per core
TOK = COLS * SEG_LEN             # 4096 tokens per core
KT_HH = 2                        # K tiles for the recurrent matmul (256 = 2*128)
G4 = 4 * HID                     # 1024 gates per direction
G8 = 2 * G4                      # both directions
F = 2 * COLS                     # free width of the [hid-tile, col] packed state

BF16 = ml_dtypes.bfloat16

_CACHE = {}


def _build_program():
    import concourse.bass as bass
    import concourse.mybir as mybir
    import concourse.tile as tile
    from concourse import bacc
    from concourse.masks import make_identity

    f32 = mybir.dt.float32
    bf16 = mybir.dt.bfloat16

    nc = bacc.Bacc("TRN2", target_bir_lowering=False, debug=False)

    # precomputed input gates: xg[l, d, p, m*COLS + n]
    xg = nc.dram_tensor("xg", [SEG_LEN, 2, 128, 8 * COLS], bf16,
                        kind="ExternalInput").ap()
    whh = nc.dram_tensor("whh", [2, 128, KT_HH * G4], bf16, kind="ExternalInput").ap()
    msk = nc.dram_tensor("msk", [128, 2], f32, kind="ExternalInput").ap()
    # raw [hid-tile, col] layout; the host does the [hid, col] -> [col, feat]
    # transpose.
    out = nc.dram_tensor("out", [2, 128, F], f32, kind="ExternalOutput").ap()
    # fwd step-6 hidden for (core 0) column 0: [128, ht] -- host-side fix
    out6 = nc.dram_tensor("out6", [128, 2], f32, kind="ExternalOutput").ap()

    with tile.TileContext(nc) as tc:
        _body(tc, bass, mybir, make_identity, xg, whh, msk, out, out6)
    nc.compile()
    return nc


def _body(tc, bass, mybir, make_identity, xg, whh, msk, out, out6):
    nc = tc.nc
    f32 = mybir.dt.float32
    bf16 = mybir.dt.bfloat16
    SIG = mybir.ActivationFunctionType.Sigmoid
    TANH = mybir.ActivationFunctionType.Tanh

    with (
        tc.tile_pool(name="singles", bufs=1) as singles,
        tc.tile_pool(name="gates", bufs=2, space="PSUM") as gp,
        tc.tile_pool(name="xgp", bufs=3) as xgp,
        tc.tile_pool(name="work", bufs=2) as work,
        tc.tile_pool(name="acts", bufs=3) as acts,
    ):
        # ---- constants / inputs to SBUF ----
        ident = singles.tile([128, 128], bf16, name="ident")
        make_identity(nc, ident)

        # input-gate blocks stream on the SP/Pool queues in use order; the
        # first (t0, d0) block is split so the i-gate slices land first
        xga = {}

        def xg_dma(t, d, q):
            l = t if d == 0 else SEG_LEN - 1 - t
            tile_ = xgp.tile([128, 8 * COLS], bf16, name=f"xga_{t}_{d}",
                             tag="xg")
            if t == 0 and d == 0:
                for ms in ((0, 2), (6, 8), (4, 6), (2, 4)):  # i, g, o, f
                    q.dma_start(
                        out=tile_[:, ms[0] * COLS:ms[1] * COLS],
                        in_=xg[l, d, :, ms[0] * COLS:ms[1] * COLS])
            else:
                q.dma_start(out=tile_, in_=xg[l, d])
            xga[(t, d)] = tile_

        xg_dma(0, 0, nc.sync)
        xg_dma(0, 1, nc.gpsimd)

        whh_sb = []
        for d in range(2):
            w2 = singles.tile([128, KT_HH * G4], bf16, name=f"whh_sb{d}")
            nc.scalar.dma_start(out=w2, in_=whh[d])
            whh_sb.append(w2)

        for t in range(1, SEG_LEN):
            xg_dma(t, 0, nc.sync)
            xg_dma(t, 1, nc.gpsimd)

        # per-core column-0 keep mask ([128, 2] = both hid tiles of col 0)
        msk_sb = singles.tile([128, 2], f32, name="msk_sb")
        nc.scalar.dma_start(out=msk_sb, in_=msk)
        Kbf = singles.tile([128, 2], bf16, name="Kbf")
        nc.vector.tensor_copy(Kbf, msk_sb)

        # ---- recurrence ----
        h_prev = [None, None]        # bf16 [128, F] per direction
        c_prev = [None, None]        # f32  [128, F] per direction
        h_fin32 = [None, None]       # final fp32 hidden per direction

        def gate_matmuls(dst, t, d, m):
            # inject the precomputed input gates, then accumulate W_hh . h
            nc.tensor.matmul(
                dst, ident,
                xga[(t, d)][:, m * COLS:(m + 1) * COLS],
                start=True,
                stop=(t == 0),
            )
            if t > 0:
                for kt in range(KT_HH):
                    nc.tensor.matmul(
                        dst,
                        whh_sb[d][:, kt * G4 + m * 128:kt * G4 + (m + 1) * 128],
                        h_prev[d][:, kt * COLS:(kt + 1) * COLS],
                        start=False,
                        stop=(kt == KT_HH - 1),
                    )

        GI, GF, GO, GG = 0, 1, 2, 3   # permuted gate row-block order i,f,o,g

        for t in range(SEG_LEN):
            for d in range(2):       # 0 = fwd, 1 = bwd
                # per-gate 2-bank PSUM units, built in i, g, f, o order so
                # each activation starts as soon as its own gate lands
                ug = {}
                for gate in (GI, GG, GF, GO):
                    if t == 0 and gate == GF:
                        continue     # sigma(f) unused at t=0
                    u = gp.tile([128, F], f32, name=f"u{t}_{d}_{gate}",
                                tag="u", bufs=4)
                    for ht in range(2):
                        gate_matmuls(u[:, ht * COLS:(ht + 1) * COLS],
                                     t, d, 2 * gate + ht)
                    ug[gate] = u

                if t == SEG_LEN - 1:
                    # ---- dedicated last step: split the activation / cell /
                    # output chain per hid-half so it overlaps the matmuls
                    # and the first output DMA ----
                    so7 = acts.tile([128, F], f32, name=f"so7_{d}", tag="so32",
                                    bufs=4)
                    tg7 = acts.tile([128, F], bf16, name=f"tg7_{d}", tag="tg")
                    hf = work.tile([128, F], f32, name=f"hfin{d}", tag="hf32",
                                   bufs=6)
                    tchs = []
                    for ht in range(2):
                        sl = slice(ht * COLS, (ht + 1) * COLS)
                        nc.scalar.activation(tg7[:, sl], ug[GG][:, sl], TANH)
                        si_h = acts.tile([128, COLS], bf16, name=f"si7_{d}_{ht}",
                                         tag="sih", bufs=4)
                        nc.scalar.activation(si_h, ug[GI][:, sl], SIG)
                        sf_h = acts.tile([128, COLS], bf16, name=f"sf7_{d}_{ht}",
                                         tag="sfh", bufs=4)
                        nc.scalar.activation(sf_h, ug[GF][:, sl], SIG)
                        t2h = work.tile([128, COLS], bf16, name=f"t27_{d}_{ht}",
                                        tag="t2", bufs=4)
                        nc.vector.tensor_mul(t2h, si_h, tg7[:, sl])
                        t1h = work.tile([128, COLS], f32, name=f"t17_{d}_{ht}",
                                        tag="t1")
                        nc.vector.tensor_mul(t1h, sf_h, c_prev[d][:, sl])
                        ch = work.tile([128, COLS], f32, name=f"c7_{d}_{ht}",
                                       tag="c7", bufs=4)
                        nc.vector.tensor_add(ch, t1h, t2h)
                        tch = acts.tile([128, COLS], f32, name=f"tc7_{d}_{ht}",
                                        tag="tc32", bufs=4)
                        nc.scalar.activation(tch, ch, TANH)
                        tchs.append(tch)
                    # sigma(o) last: o's PSUM lands last, so the cell chains
                    # above never queue behind it on the Act engine
                    for ht in range(2):
                        sl = slice(ht * COLS, (ht + 1) * COLS)
                        nc.scalar.activation(so7[:, sl], ug[GO][:, sl], SIG)
                        nc.vector.tensor_mul(hf[:, sl], so7[:, sl], tchs[ht])
                        nc.sync.dma_start(out=out[d, :, sl], in_=hf[:, sl])
                    h_fin32[d] = hf
                    continue

                # step-6 fwd activations stay fp32 (they feed the out6 fix)
                adt = f32 if t == SEG_LEN - 2 else bf16
                si = acts.tile([128, F], bf16, name=f"si_{t}_{d}", tag="si")
                nc.scalar.activation(si, ug[GI], SIG)
                # tanh(g) next: it gates the critical path to h via t2
                tg = acts.tile([128, F], bf16, name=f"tg_{t}_{d}", tag="tg")
                nc.scalar.activation(tg, ug[GG], TANH)
                if t > 0:
                    sf = acts.tile([128, F], bf16, name=f"sf_{t}_{d}", tag="sf")
                    nc.scalar.activation(sf, ug[GF], SIG)
                so = acts.tile([128, F], adt, name=f"so_{t}_{d}",
                               tag="so32" if adt is f32 else "so", bufs=4)
                nc.scalar.activation(so, ug[GO], SIG)

                # cell update: c (fp32) = sig_f * c + sig_i * tanh_g
                t2 = work.tile([128, F], bf16, name=f"t2_{t}_{d}", tag="t2",
                               bufs=4)
                nc.vector.tensor_mul(t2, si, tg)
                if t == 0:
                    c_new = t2
                else:
                    t1 = work.tile([128, F], f32, name=f"t1_{t}_{d}", tag="t1")
                    nc.vector.tensor_mul(t1, sf, c_prev[d])
                    c_new = work.tile([128, F], f32, name=f"c_{t}_{d}", tag=f"c{d}")
                    nc.vector.tensor_add(c_new, t1, t2)

                tc_ = acts.tile([128, F], adt, name=f"tc_{t}_{d}",
                                tag="tc32" if adt is f32 else "tc", bufs=4)
                h_bf = work.tile([128, F], bf16, name=f"h_{t}_{d}", tag=f"h{d}")
                if t == 0:
                    # split per hid-half: the t=1 hh matmul on k-tile 0 only
                    # needs h[:, 0:COLS], so let it start one op earlier
                    for ht in range(2):
                        sl = slice(ht * COLS, (ht + 1) * COLS)
                        nc.scalar.activation(tc_[:, sl], c_new[:, sl], TANH)
                        nc.vector.tensor_mul(h_bf[:, sl], so[:, sl], tc_[:, sl])
                else:
                    nc.scalar.activation(tc_, c_new, TANH)
                    nc.vector.tensor_mul(h_bf, so, tc_)

                if d == 1 and t == 0:
                    # bwd step 0 is masked for (core 0) column 0: zero the
                    # col-0 slice of h, c in place (per-core mask data)
                    c0 = c_new[:, 0:F:COLS]
                    nc.vector.tensor_mul(c0, c0, msk_sb)
                    h0 = h_bf[:, 0:F:COLS]
                    nc.vector.tensor_mul(h0, h0, Kbf)

                if d == 0 and t == SEG_LEN - 2:
                    # fwd h after step 6, col 0 only (ragged col-0, host fix)
                    h6c = work.tile([128, 2], f32, name="h6c", tag="h6c")
                    nc.vector.tensor_mul(h6c, so[:, 0:F:COLS], tc_[:, 0:F:COLS])
                    nc.gpsimd.dma_start(out=out6, in_=h6c)

                c_prev[d] = c_new
                h_prev[d] = h_bf

        # (outputs are DMA'd inside the loop; the host applies the core-0
        # column-0 ragged fix from out6)


def _prep_weights(inputs):
    """Per-vocab input-gates table (bf16) + gate-permuted recurrent weights.
    Cached on the identity of the input arrays."""
    key = tuple(id(inputs[k]) for k in
                ("emb_table", "w_ih_f", "w_hh_f", "b_ih_f", "b_hh_f",
                 "w_ih_b", "w_hh_b", "b_ih_b", "b_hh_b"))
    hit = _CACHE.get("weights")
    if hit is not None and hit[0] == key:
        return hit[2]
    # keep strong refs to the inputs so ids cannot be recycled
    refs = [inputs[k] for k in
            ("emb_table", "w_ih_f", "w_hh_f", "b_ih_f", "b_hh_f",
             "w_ih_b", "w_hh_b", "b_ih_b", "b_hh_b")]

    perm = np.concatenate([np.arange(0, 2 * HID),            # i, f
                           np.arange(3 * HID, 4 * HID),      # o
                           np.arange(2 * HID, 3 * HID)])     # g

    def aug(w_ih, b_ih, b_hh):
        a = np.empty((G4, EMBED + 1), dtype=np.float32)
        a[:, :EMBED] = np.asarray(w_ih, np.float32)
        a[:, EMBED] = np.asarray(b_ih, np.float32) + np.asarray(b_hh, np.float32)
        return a[perm]

    AUG = np.concatenate(
        [aug(inputs["w_ih_f"], inputs["b_ih_f"], inputs["b_hh_f"]),
         aug(inputs["w_ih_b"], inputs["b_ih_b"], inputs["b_hh_b"])])  # [G8, 301]
    E1 = np.empty((VOCAB, EMBED + 1), dtype=np.float32)
    E1[:, :EMBED] = np.asarray(inputs["emb_table"], np.float32)
    E1[:, EMBED] = 1.0
    # the per-vocab gates table: one exact fp32 GEMM, rounded once to bf16
    GT = (E1 @ AUG.T).astype(BF16)                            # [VOCAB, G8]

    def prep_whh(w_hh):
        a = np.asarray(w_hh, np.float32)[perm].T.reshape(KT_HH, 128, G4)
        return np.ascontiguousarray(
            a.transpose(1, 0, 2).reshape(128, KT_HH * G4)).astype(BF16)

    whh_arr = np.stack([prep_whh(inputs["w_hh_f"]), prep_whh(inputs["w_hh_b"])])

    # keep mask for column 0 (both hid tiles): 0 on core 0, 1 elsewhere
    msk_plain = np.ones((128, 2), dtype=np.float32)
    msk_core0 = np.zeros((128, 2), dtype=np.float32)

    prepped = (GT, whh_arr, msk_core0, msk_plain)
    _CACHE["weights"] = (key, refs, prepped)
    return prepped


def _tokens_per_core(seq):
    """v[c][l, n] = token id for (core c, step l, column n)."""
    vs = []
    for c in range(NCORES):
        if c == 0:
            w = np.concatenate([seq[0:1], seq[0:TOK - 1]])
        else:
            w = seq[TOK * c - 1: TOK * c + TOK - 1]
        v = w.reshape(COLS, SEG_LEN).T.copy()   # v[l, n]
        if c == 0:
            v[:, 0] = seq[0:SEG_LEN]            # col 0: seq[0..7], step 7 masked
        vs.append(v)
    return np.stack(vs)                          # [NCORES, SEG_LEN, COLS]


def _prep_host(inputs):
    """Build the per-core input maps from the full problem inputs."""
    GT, whh_arr, msk_core0, msk_plain = _prep_weights(inputs)
    seq = np.asarray(inputs["seq_s"]).astype(np.int64)

    V = _tokens_per_core(seq)                    # [8, L, N]
    TG = GT[V]                                   # [8, L, N, G8] bf16
    # xg[c][l, d, p, m*COLS + n] = TG[c, l, n, d*1024 + m*128 + p]
    X = np.ascontiguousarray(
        TG.reshape(NCORES, SEG_LEN, COLS, 2, 8, 128)
          .transpose(0, 1, 3, 5, 4, 2)
    ).reshape(NCORES, SEG_LEN, 2, 128, 8 * COLS)

    in_maps = []
    for c in range(NCORES):
        in_maps.append({
            "xg": X[c],
            "whh": whh_arr,
            "msk": msk_core0 if c == 0 else msk_plain,
        })
    return in_maps


def kernel(**inputs) -> np.ndarray:
    from concourse import bass_utils

    if "nc" not in _CACHE:
        _CACHE["nc"] = _build_program()
    nc = _CACHE["nc"]

    in_maps = _prep_host(inputs)
    res = bass_utils.run_bass_kernel_spmd(nc, in_maps, core_ids=list(range(NCORES)))
    return np.concatenate(
        [_untangle_out(r["out"], r["out6"] if c == 0 else None)
         for c, r in enumerate(res.results)], axis=0)


def _untangle_out(o, o6=None):
    """Device out [2, 128, F] -> [COLS, 2*HID]:
    o[d, p, ht*COLS + n] = h[d][hid = ht*128 + p, col n].
    o6 (core 0 only): [128, ht] = fwd step-6 hidden of column 0, which
    replaces the fwd features of the ragged column 0."""
    o4 = np.asarray(o).reshape(2, 128, 2, COLS)
    r = np.ascontiguousarray(o4.transpose(3, 0, 2, 1)).reshape(COLS, 2 * HID)
    if o6 is not None:
        r[0, :HID] = np.asarray(o6).T.reshape(HID)
    return r


if __name__ == "__main__":
    nc = _build_program()
    print("program built ok")


# revision 71
# speedup vs baseline: 1.1648x; 1.1648x over previous
"""Trainium2 Bass kernel for nn_ColumnEncoding (bidirectional masked LSTM
over 4096 split-delimited token segments).

Sharding: data-parallel over the 4096 columns -> 512 columns per core on 8
cores.  LSTM weights are replicated.  Each core runs an identical SPMD Bass
program on its shard; the host concatenates the 8 [512, 512] outputs.

The input-side gate pre-activations W_in . E[tok] + b depend only on the
token id, so the host precomputes a per-vocab gates table once per weight
set (one 39-GFLOP BLAS matmul, cached) and per call just gathers rows for
the 32768 tokens.  The device receives the input gates pre-computed in bf16
and injects each [128, 512] block into PSUM with a single identity matmul
(512 PE cycles instead of the 3x512 the three K-tile input matmuls cost),
then accumulates only the recurrent W_hh . h_{t-1} matmuls on top.  This
cuts TensorE time ~40% below the classic dataflow; the ScalarE activation
wall becomes the bound.

Per-core device pipeline:
  1. Input-gate blocks ([128, 8*512] bf16 per (step, direction)) stream in
     on two DMA queues in use order; recurrent weights and the column mask
     load on the Act queue.  The first matmul starts ~2.4us in.
  2. For each step t (8) and direction (fwd l=t / bwd l=7-t), gates^T
     [1024, 512cols] are built in four 2-bank per-gate PSUM units (i, g,
     f, o order) as identity-matmul injection of the precomputed input
     gates + W_hh^T @ h_{t-1} (2 K-tiles, skipped at t=0).
  3. ScalarE applies sigmoid/tanh per gate (bf16 outputs); VectorE does the
     cell update (c in fp32, products in bf16 for the 2x/4x DVE modes); h is
     written bf16 and fed back as the next matmul rhs.
  4. The ragged first column (segment length 7 instead of 8) is handled with
     tiny column-0 masked ops ([128, 2] slices, per-core mask data) and a
     host-side fix from the step-6 hidden (out6), keeping the program SPMD.
  5. Final hidden states stream out per hid-half during the last step; the
     host untangles [hid, col] -> [col, feat].
"""

import numpy as np
import ml_dtypes

VOCAB = 32000
EMBED = 300
HID = 256
N_COLS = 4096
SEG_LEN = 8
T = N_COLS * SEG_LEN
NCORES = 8
COLS = N_COLS // NCORES          # 512 columns per core
TOK = COLS * SEG_LEN             # 4096 tokens per core
KT_HH = 2                        # K tiles for the recurrent matmul (256 = 2*128)
G4 = 4 * HID                     # 1024 gates per direction
G8 = 2 * G4                      # both directions
F = 2 * COLS                     # free width of the [hid-tile, col] packed state

BF16 = ml_dtypes.bfloat16

_CACHE = {}


def _build_program():
    import concourse.bass as bass
    import concourse.mybir as mybir
    import concourse.tile as tile
    from concourse import bacc
    from concourse.masks import make_identity

    f32 = mybir.dt.float32
    bf16 = mybir.dt.bfloat16

    nc = bacc.Bacc("TRN2", target_bir_lowering=False, debug=False)

    # precomputed input gates: xg[l, d, p, m*COLS + n]
    xg = nc.dram_tensor("xg", [SEG_LEN, 2, 128, 8 * COLS], bf16,
                        kind="ExternalInput").ap()
    whh = nc.dram_tensor("whh", [2, 128, KT_HH * G4], bf16, kind="ExternalInput").ap()
    msk = nc.dram_tensor("msk", [128, 2], f32, kind="ExternalInput").ap()
    # raw [hid-tile, col] layout; the host does the [hid, col] -> [col, feat]
    # transpose.
    out = nc.dram_tensor("out", [2, 128, F], f32, kind="ExternalOutput").ap()
    # fwd step-6 hidden for (core 0) column 0: [128, ht] -- host-side fix
    out6 = nc.dram_tensor("out6", [128, 2], f32, kind="ExternalOutput").ap()

    with tile.TileContext(nc) as tc:
        _body(tc, bass, mybir, make_identity, xg, whh, msk, out, out6)
    nc.compile()
    return nc


def _body(tc, bass, mybir, make_identity, xg, whh, msk, out, out6):
    nc = tc.nc
    f32 = mybir.dt.float32
    bf16 = mybir.dt.bfloat16
    SIG = mybir.ActivationFunctionType.Sigmoid
    TANH = mybir.ActivationFunctionType.Tanh

    with (
        tc.tile_pool(name="singles", bufs=1) as singles,
        tc.tile_pool(name="gates", bufs=2, space="PSUM") as gp,
        tc.tile_pool(name="xgp", bufs=3) as xgp,
        tc.tile_pool(name="work", bufs=2) as work,
        tc.tile_pool(name="acts", bufs=3) as acts,
    ):
        # ---- constants / inputs to SBUF ----
        ident = singles.tile([128, 128], bf16, name="ident")
        make_identity(nc, ident)

        # input-gate blocks stream on the SP/Pool queues in use order; the
        # first (t0, d0) block is split so the i-gate slices land first
        xga = {}

        def xg_dma(t, d, q):
            l = t if d == 0 else SEG_LEN - 1 - t
            tile_ = xgp.tile([128, 8 * COLS], bf16, name=f"xga_{t}_{d}",
                             tag="xg")
            if t == 0 and d == 0:
                for ms in ((0, 2), (6, 8), (4, 6), (2, 4)):  # i, g, o, f
                    q.dma_start(
                        out=tile_[:, ms[0] * COLS:ms[1] * COLS],
                        in_=xg[l, d, :, ms[0] * COLS:ms[1] * COLS])
            else:
                q.dma_start(out=tile_, in_=xg[l, d])
            xga[(t, d)] = tile_

        xg_dma(0, 0, nc.sync)
        xg_dma(0, 1, nc.gpsimd)

        whh_sb = []
        for d in range(2):
            w2 = singles.tile([128, KT_HH * G4], bf16, name=f"whh_sb{d}")
            nc.scalar.dma_start(out=w2, in_=whh[d])
            whh_sb.append(w2)

        for t in range(1, SEG_LEN):
            xg_dma(t, 0, nc.sync)
            xg_dma(t, 1, nc.gpsimd)

        # per-core column-0 keep mask ([128, 2] = both hid tiles of col 0)
        msk_sb = singles.tile([128, 2], f32, name="msk_sb")
        nc.scalar.dma_start(out=msk_sb, in_=msk)
        Kbf = singles.tile([128, 2], bf16, name="Kbf")
        nc.vector.tensor_copy(Kbf, msk_sb)

        # ---- recurrence ----
        h_prev = [None, None]        # bf16 [128, F] per direction
        c_prev = [None, None]        # f32  [128, F] per direction
        h_fin32 = [None, None]       # final fp32 hidden per direction

        def gate_matmuls(dst, t, d, m):
            # inject the precomputed input gates, then accumulate W_hh . h
            nc.tensor.matmul(
                dst, ident,
                xga[(t, d)][:, m * COLS:(m + 1) * COLS],
                start=True,
                stop=(t == 0),
            )
            if t > 0:
                for kt in range(KT_HH):
                    nc.tensor.matmul(
                        dst,
                        whh_sb[d][:, kt * G4 + m * 128:kt * G4 + (m + 1) * 128],
                        h_prev[d][:, kt * COLS:(kt + 1) * COLS],
                        start=False,
                        stop=(kt == KT_HH - 1),
                    )

        GI, GF, GO, GG = 0, 1, 2, 3   # permuted gate row-block order i,f,o,g

        for t in range(SEG_LEN):
            for d in range(2):       # 0 = fwd, 1 = bwd
                # per-gate 2-bank PSUM units, built in i, g, f, o order so
                # each activation starts as soon as its own gate lands
                ug = {}
                for gate in (GI, GG, GF, GO):
                    if t == 0 and gate == GF:
                        continue     # sigma(f) unused at t=0
                    u = gp.tile([128, F], f32, name=f"u{t}_{d}_{gate}",
                                tag="u", bufs=4)
                    for ht in range(2):
                        gate_matmuls(u[:, ht * COLS:(ht + 1) * COLS],
                                     t, d, 2 * gate + ht)
                    ug[gate] = u

                if t == SEG_LEN - 1:
                    # ---- dedicated last step: split the activation / cell /
                    # output chain per hid-half so it overlaps the matmuls
                    # and the first output DMA ----
                    so7 = acts.tile([128, F], f32, name=f"so7_{d}", tag="so32",
                                    bufs=4)
                    tg7 = acts.tile([128, F], bf16, name=f"tg7_{d}", tag="tg")
                    hf = work.tile([128, F], f32, name=f"hfin{d}", tag="hf32",
                                   bufs=6)
                    tchs = []
                    for ht in range(2):
                        sl = slice(ht * COLS, (ht + 1) * COLS)
                        nc.scalar.activation(tg7[:, sl], ug[GG][:, sl], TANH)
                        si_h = acts.tile([128, COLS], bf16, name=f"si7_{d}_{ht}",
                                         tag="sih", bufs=4)
                        nc.scalar.activation(si_h, ug[GI][:, sl], SIG)
                        sf_h = acts.tile([128, COLS], bf16, name=f"sf7_{d}_{ht}",
                                         tag="sfh", bufs=4)
                        nc.scalar.activation(sf_h, ug[GF][:, sl], SIG)
                        t2h = work.tile([128, COLS], bf16, name=f"t27_{d}_{ht}",
                                        tag="t2", bufs=4)
                        nc.vector.tensor_mul(t2h, si_h, tg7[:, sl])
                        t1h = work.tile([128, COLS], f32, name=f"t17_{d}_{ht}",
                                        tag="t1")
                        nc.vector.tensor_mul(t1h, sf_h, c_prev[d][:, sl])
                        ch = work.tile([128, COLS], f32, name=f"c7_{d}_{ht}",
                                       tag="c7", bufs=4)
                        nc.vector.tensor_add(ch, t1h, t2h)
                        tch = acts.tile([128, COLS], f32, name=f"tc7_{d}_{ht}",
                                        tag="tc32", bufs=4)
                        nc.scalar.activation(tch, ch, TANH)
                        tchs.append(tch)
                    # sigma(o) last: o's PSUM lands last, so the cell chains
                    # above never queue behind it on the Act engine
                    for ht in range(2):
                        sl = slice(ht * COLS, (ht + 1) * COLS)
                        nc.scalar.activation(so7[:, sl], ug[GO][:, sl], SIG)
                        nc.vector.tensor_mul(hf[:, sl], so7[:, sl], tchs[ht])
                        nc.sync.dma_start(out=out[d, :, sl], in_=hf[:, sl])
                    h_fin32[d] = hf
                    continue

                # step-6 fwd activations stay fp32 (they feed the out6 fix)
                adt = f32 if t == SEG_LEN - 2 else bf16
                si = acts.tile([128, F], bf16, name=f"si_{t}_{d}", tag="si")
                nc.scalar.activation(si, ug[GI], SIG)
                # tanh(g) next: it gates the critical path to h via t2
                tg = acts.tile([128, F], bf16, name=f"tg_{t}_{d}", tag="tg")
                nc.scalar.activation(tg, ug[GG], TANH)
                if t > 0:
                    sf = acts.tile([128, F], bf16, name=f"sf_{t}_{d}", tag="sf")
                    nc.scalar.activation(sf, ug[GF], SIG)
                so = acts.tile([128, F], adt, name=f"so_{t}_{d}",
                               tag="so32" if adt is f32 else "so", bufs=4)
                nc.scalar.activation(so, ug[GO], SIG)

                # cell update: c (fp32) = sig_f * c + sig_i * tanh_g
                t2 = work.tile([128, F], bf16, name=f"t2_{t}_{d}", tag="t2",
                               bufs=4)
                nc.vector.tensor_mul(t2, si, tg)
                if t == 0:
                    c_new = t2
                else:
                    t1 = work.tile([128, F], f32, name=f"t1_{t}_{d}", tag="t1")
                    nc.vector.tensor_mul(t1, sf, c_prev[d])
                    c_new = work.tile([128, F], f32, name=f"c_{t}_{d}", tag=f"c{d}")
                    nc.vector.tensor_add(c_new, t1, t2)

                tc_ = acts.tile([128, F], adt, name=f"tc_{t}_{d}",
                                tag="tc32" if adt is f32 else "tc", bufs=4)
                h_bf = work.tile([128, F], bf16, name=f"h_{t}_{d}", tag=f"h{d}")
                if t == 0:
                    # split per hid-half: the t=1 hh matmul on k-tile 0 only
                    # needs h[:, 0:COLS], so let it start one op earlier
                    for ht in range(2):
                        sl = slice(ht * COLS, (ht + 1) * COLS)
                        nc.scalar.activation(tc_[:, sl], c_new[:, sl], TANH)
                        nc.vector.tensor_mul(h_bf[:, sl], so[:, sl], tc_[:, sl])
                else:
                    nc.scalar.activation(tc_, c_new, TANH)
                    nc.vector.tensor_mul(h_bf, so, tc_)

                if d == 1 and t == 0:
                    # bwd step 0 is masked for (core 0) column 0: zero the
                    # col-0 slice of h, c in place (per-core mask data)
                    c0 = c_new[:, 0:F:COLS]
                    nc.vector.tensor_mul(c0, c0, msk_sb)
                    h0 = h_bf[:, 0:F:COLS]
                    nc.vector.tensor_mul(h0, h0, Kbf)

                if d == 0 and t == SEG_LEN - 2:
                    # fwd h after step 6, col 0 only (ragged col-0, host fix)
                    h6c = work.tile([128, 2], f32, name="h6c", tag="h6c")
                    nc.vector.tensor_mul(h6c, so[:, 0:F:COLS], tc_[:, 0:F:COLS])
                    nc.gpsimd.dma_start(out=out6, in_=h6c)

                c_prev[d] = c_new
                h_prev[d] = h_bf

        # (outputs are DMA'd inside the loop; the host applies the core-0
        # column-0 ragged fix from out6)


def _prep_weights(inputs):
    """Per-vocab input-gates table (bf16) + gate-permuted recurrent weights.
    Cached on the identity of the input arrays."""
    key = tuple(id(inputs[k]) for k in
                ("emb_table", "w_ih_f", "w_hh_f", "b_ih_f", "b_hh_f",
                 "w_ih_b", "w_hh_b", "b_ih_b", "b_hh_b"))
    hit = _CACHE.get("weights")
    if hit is not None and hit[0] == key:
        return hit[2]
    # keep strong refs to the inputs so ids cannot be recycled
    refs = [inputs[k] for k in
            ("emb_table", "w_ih_f", "w_hh_f", "b_ih_f", "b_hh_f",
             "w_ih_b", "w_hh_b", "b_ih_b", "b_hh_b")]

    perm = np.concatenate([np.arange(0, 2 * HID),            # i, f
                           np.arange(3 * HID, 4 * HID),      # o
                           np.arange(2 * HID, 3 * HID)])     # g

    def aug(w_ih, b_ih, b_hh):
        a = np.empty((G4, EMBED + 1), dtype=np.float32)
        a[:, :EMBED] = np.asarray(w_ih, np.float32)
        a[:, EMBED] = np.asarray(b_ih, np.float32) + np.asarray(b_hh, np.float32)
        return a[perm]

    AUG = np.concatenate(
        [aug(inputs["w_ih_f"], inputs["b_ih_f"], inputs["b_hh_f"]),
         aug(inputs["w_ih_b"], inputs["b_ih_b"], inputs["b_hh_b"])])  # [G8, 301]
    E1 = np.empty((VOCAB, EMBED + 1), dtype=np.float32)
    E1[:, :EMBED] = np.asarray(inputs["emb_table"], np.float32)
    E1[:, EMBED] = 1.0
    # the per-vocab gates table: one exact fp32 GEMM, rounded once to bf16
    GT = (E1 @ AUG.T).astype(BF16)                            # [VOCAB, G8]

    def prep_whh(w_hh):
        a = np.asarray(w_hh, np.float32)[perm].T.reshape(KT_HH, 128, G4)
        return np.ascontiguousarray(
            a.transpose(1, 0, 2).reshape(128, KT_HH * G4)).astype(BF16)

    whh_arr = np.stack([prep_whh(inputs["w_hh_f"]), prep_whh(inputs["w_hh_b"])])

    # keep mask for column 0 (both hid tiles): 0 on core 0, 1 elsewhere
    msk_plain = np.ones((128, 2), dtype=np.float32)
    msk_core0 = np.zeros((128, 2), dtype=np.float32)

    prepped = (GT, whh_arr, msk_core0, msk_plain)
    _CACHE["weights"] = (key, refs, prepped)
    return prepped


def _tokens_per_core(seq):
    """v[c][l, n] = token id for (core c, step l, column n)."""
    vs = []
    for c in range(NCORES):
        if c == 0:
            w = np.concatenate([seq[0:1], seq[0:TOK - 1]])
        else:
            w = seq[TOK * c - 1: TOK * c + TOK - 1]
        v = w.reshape(COLS, SEG_LEN).T.copy()   # v[l, n]
        if c == 0:
            v[:, 0] = seq[0:SEG_LEN]            # col 0: seq[0..7], step 7 masked
        vs.append(v)
    return np.stack(vs)                          # [NCORES, SEG_LEN, COLS]


def _prep_host(inputs):
    """Build the per-core input maps from the full problem inputs."""
    GT, whh_arr, msk_core0, msk_plain = _prep_weights(inputs)
    seq = np.asarray(inputs["seq_s"]).astype(np.int64)

    V = _tokens_per_core(seq)                    # [8, L, N]
    TG = GT[V]                                   # [8, L, N, G8] bf16
    # xg[c][l, d, p, m*COLS + n] = TG[c, l, n, d*1024 + m*128 + p]
    X = np.ascontiguousarray(
        TG.reshape(NCORES, SEG_LEN, COLS, 2, 8, 128)
          .transpose(0, 1, 3, 5, 4, 2)
    ).reshape(NCORES, SEG_LEN, 2, 128, 8 * COLS)

    in_maps = []
    for c in range(NCORES):
        in_maps.append({
            "xg": X[c],
            "whh": whh_arr,
            "msk": msk_core0 if c == 0 else msk_plain,
        })
    return in_maps


def kernel(**inputs) -> np.ndarray:
    from concourse import bass_utils

    if "nc" not in _CACHE:
        _CACHE["nc"] = _build_program()
    nc = _CACHE["nc"]

    in_maps = _prep_host(inputs)
    res = bass_utils.run_bass_kernel_spmd(nc, in_maps, core_ids=list(range(NCORES)))
    return np.concatenate(
        [_untangle_out(r["out"], r["out6"] if c == 0 else None)
         for c, r in enumerate(res.results)], axis=0)


def _untangle_out(o, o6=None):
    """Device out [2, 128, F] -> [COLS, 2*HID]:
    o[d, p, ht*COLS + n] = h[d][hid = ht*128 + p, col n].
    o6 (core 0 only): [128, ht] = fwd step-6 hidden of column 0, which
    replaces the fwd features of the ragged column 0."""
    o4 = np.asarray(o).reshape(2, 128, 2, COLS)
    r = np.ascontiguousarray(o4.transpose(3, 0, 2, 1)).reshape(COLS, 2 * HID)
    if o6 is not None:
        r[0, :HID] = np.asarray(o6).T.reshape(HID)
    return r


if __name__ == "__main__":
    nc = _build_program()
    print("program built ok")


# revision 72
# speedup vs baseline: 1.2338x; 1.0592x over previous
"""Trainium2 Bass kernel for nn_ColumnEncoding (bidirectional masked LSTM
over 4096 split-delimited token segments).

Sharding: data-parallel over the 4096 columns -> 512 columns per core on 8
cores.  LSTM weights are replicated.  Each core runs an identical SPMD Bass
program on its shard; the host concatenates the 8 [512, 512] outputs.

The input-side gate pre-activations W_in . E[tok] + b depend only on the
token id, so the host precomputes a per-vocab gates table once per weight
set (one 39-GFLOP BLAS matmul, cached) and per call just gathers rows for
the 32768 tokens.  The device receives the input gates pre-computed in bf16
and injects each [128, 512] block into PSUM with a single identity matmul
(512 PE cycles instead of the 3x512 the three K-tile input matmuls cost),
then accumulates only the recurrent W_hh . h_{t-1} matmuls on top.  This
cuts TensorE time ~40% below the classic dataflow; the ScalarE activation
wall becomes the bound.

Per-core device pipeline:
  1. Input-gate blocks ([128, 8*512] bf16 per (step, direction)) stream in
     on two DMA queues in use order; recurrent weights and the column mask
     load on the Act queue.  The first matmul starts ~2.4us in.
  2. For each step t (8) and direction (fwd l=t / bwd l=7-t), gates^T
     [1024, 512cols] are built in four 2-bank per-gate PSUM units (i, g,
     f, o order) as identity-matmul injection of the precomputed input
     gates + W_hh^T @ h_{t-1} (2 K-tiles, skipped at t=0).
  3. ScalarE applies sigmoid/tanh per gate (bf16 outputs); VectorE does the
     cell update (c in fp32, products in bf16 for the 2x/4x DVE modes); h is
     written bf16 and fed back as the next matmul rhs.
  4. The ragged first column (segment length 7 instead of 8) is handled with
     tiny column-0 masked ops ([128, 2] slices, per-core mask data) and a
     host-side fix from the step-6 hidden (out6), keeping the program SPMD.
  5. Final hidden states stream out per hid-half during the last step; the
     host untangles [hid, col] -> [col, feat].
"""

import numpy as np
import ml_dtypes

VOCAB = 32000
EMBED = 300
HID = 256
N_COLS = 4096
SEG_LEN = 8
T = N_COLS * SEG_LEN
NCORES = 8
COLS = N_COLS // NCORES          # 512 columns per core
TOK = COLS * SEG_LEN             # 4096 tokens per core
KT_HH = 2                        # K tiles for the recurrent matmul (256 = 2*128)
G4 = 4 * HID                     # 1024 gates per direction
G8 = 2 * G4                      # both directions
F = 2 * COLS                     # free width of the [hid-tile, col] packed state

BF16 = ml_dtypes.bfloat16

_CACHE = {}


def _build_program():
    import concourse.bass as bass
    import concourse.mybir as mybir
    import concourse.tile as tile
    from concourse import bacc
    from concourse.masks import make_identity

    f32 = mybir.dt.float32
    bf16 = mybir.dt.bfloat16

    nc = bacc.Bacc("TRN2", target_bir_lowering=False, debug=False)

    # precomputed input gates: xg[l, d, p, m*COLS + n]
    xg = nc.dram_tensor("xg", [SEG_LEN, 2, 128, 8 * COLS], bf16,
                        kind="ExternalInput").ap()
    whh = nc.dram_tensor("whh", [2, 128, KT_HH * G4], bf16, kind="ExternalInput").ap()
    msk = nc.dram_tensor("msk", [128, 2], f32, kind="ExternalInput").ap()
    # raw [hid-tile, col] layout; the host does the [hid, col] -> [col, feat]
    # transpose.
    out = nc.dram_tensor("out", [2, 128, F], f32, kind="ExternalOutput").ap()
    # fwd step-6 hidden for (core 0) column 0: [128, ht] -- host-side fix
    out6 = nc.dram_tensor("out6", [128, 2], f32, kind="ExternalOutput").ap()

    with tile.TileContext(nc) as tc:
        _body(tc, bass, mybir, make_identity, xg, whh, msk, out, out6)
    nc.compile()
    return nc


def _body(tc, bass, mybir, make_identity, xg, whh, msk, out, out6):
    nc = tc.nc
    f32 = mybir.dt.float32
    bf16 = mybir.dt.bfloat16
    SIG = mybir.ActivationFunctionType.Sigmoid
    TANH = mybir.ActivationFunctionType.Tanh

    with (
        tc.tile_pool(name="singles", bufs=1) as singles,
        tc.tile_pool(name="gates", bufs=2, space="PSUM") as gp,
        tc.tile_pool(name="xgp", bufs=3) as xgp,
        tc.tile_pool(name="work", bufs=2) as work,
        tc.tile_pool(name="acts", bufs=3) as acts,
    ):
        # ---- constants / inputs to SBUF ----
        ident = singles.tile([128, 128], bf16, name="ident")
        make_identity(nc, ident)

        # input-gate blocks stream on the SP/Pool queues in use order; the
        # first (t0, d0) block is split so the i-gate slices land first
        xga = {}

        def xg_dma(t, d, q):
            l = t if d == 0 else SEG_LEN - 1 - t
            tile_ = xgp.tile([128, 8 * COLS], bf16, name=f"xga_{t}_{d}",
                             tag="xg")
            if t == 0 and d == 0:
                for ms in ((0, 2), (6, 8), (4, 6), (2, 4)):  # i, g, o, f
                    q.dma_start(
                        out=tile_[:, ms[0] * COLS:ms[1] * COLS],
                        in_=xg[l, d, :, ms[0] * COLS:ms[1] * COLS])
            else:
                q.dma_start(out=tile_, in_=xg[l, d])
            xga[(t, d)] = tile_

        xg_dma(0, 0, nc.sync)
        xg_dma(0, 1, nc.gpsimd)

        whh_sb = []
        for d in range(2):
            w2 = singles.tile([128, KT_HH * G4], bf16, name=f"whh_sb{d}")
            nc.scalar.dma_start(out=w2, in_=whh[d])
            whh_sb.append(w2)

        for t in range(1, SEG_LEN):
            xg_dma(t, 0, nc.sync)
            xg_dma(t, 1, nc.gpsimd)

        # per-core column-0 keep mask ([128, 2] = both hid tiles of col 0)
        msk_sb = singles.tile([128, 2], f32, name="msk_sb")
        nc.scalar.dma_start(out=msk_sb, in_=msk)
        Kbf = singles.tile([128, 2], bf16, name="Kbf")
        nc.vector.tensor_copy(Kbf, msk_sb)

        # ---- recurrence ----
        h_prev = [None, None]        # bf16 [128, F] per direction
        c_prev = [None, None]        # f32  [128, F] per direction
        h_fin32 = [None, None]       # final fp32 hidden per direction

        def gate_matmuls(dst, t, d, m):
            # inject the precomputed input gates, then accumulate W_hh . h
            nc.tensor.matmul(
                dst, ident,
                xga[(t, d)][:, m * COLS:(m + 1) * COLS],
                start=True,
                stop=(t == 0),
            )
            if t > 0:
                for kt in range(KT_HH):
                    nc.tensor.matmul(
                        dst,
                        whh_sb[d][:, kt * G4 + m * 128:kt * G4 + (m + 1) * 128],
                        h_prev[d][:, kt * COLS:(kt + 1) * COLS],
                        start=False,
                        stop=(kt == KT_HH - 1),
                    )

        GI, GF, GO, GG = 0, 1, 2, 3   # permuted gate row-block order i,f,o,g

        for t in range(SEG_LEN):
            for d in range(2):       # 0 = fwd, 1 = bwd
                # per-gate 2-bank PSUM units, built in i, g, f, o order so
                # each activation starts as soon as its own gate lands
                ug = {}
                for gate in (GI, GG, GF, GO):
                    if t == 0 and gate == GF:
                        continue     # sigma(f) unused at t=0
                    u = gp.tile([128, F], f32, name=f"u{t}_{d}_{gate}",
                                tag="u", bufs=4)
                    for ht in range(2):
                        gate_matmuls(u[:, ht * COLS:(ht + 1) * COLS],
                                     t, d, 2 * gate + ht)
                    ug[gate] = u

                if t == SEG_LEN - 1:
                    # ---- dedicated last step: split the activation / cell /
                    # output chain per hid-half so it overlaps the matmuls
                    # and the first output DMA ----
                    so7 = acts.tile([128, F], f32, name=f"so7_{d}", tag="so32",
                                    bufs=4)
                    tg7 = acts.tile([128, F], bf16, name=f"tg7_{d}", tag="tg")
                    hf = work.tile([128, F], f32, name=f"hfin{d}", tag="hf32",
                                   bufs=6)
                    tchs = []
                    for ht in range(2):
                        sl = slice(ht * COLS, (ht + 1) * COLS)
                        nc.scalar.activation(tg7[:, sl], ug[GG][:, sl], TANH)
                        si_h = acts.tile([128, COLS], bf16, name=f"si7_{d}_{ht}",
                                         tag="sih", bufs=4)
                        nc.scalar.activation(si_h, ug[GI][:, sl], SIG)
                        sf_h = acts.tile([128, COLS], bf16, name=f"sf7_{d}_{ht}",
                                         tag="sfh", bufs=4)
                        nc.scalar.activation(sf_h, ug[GF][:, sl], SIG)
                        t2h = work.tile([128, COLS], bf16, name=f"t27_{d}_{ht}",
                                        tag="t2", bufs=4)
                        nc.vector.tensor_mul(t2h, si_h, tg7[:, sl])
                        t1h = work.tile([128, COLS], f32, name=f"t17_{d}_{ht}",
                                        tag="t1")
                        nc.vector.tensor_mul(t1h, sf_h, c_prev[d][:, sl])
                        ch = work.tile([128, COLS], f32, name=f"c7_{d}_{ht}",
                                       tag="c7", bufs=4)
                        nc.vector.tensor_add(ch, t1h, t2h)
                        tch = acts.tile([128, COLS], f32, name=f"tc7_{d}_{ht}",
                                        tag="tc32", bufs=4)
                        nc.scalar.activation(tch, ch, TANH)
                        tchs.append(tch)
                    # sigma(o) last: o's PSUM lands last, so the cell chains
                    # above never queue behind it on the Act engine.  The
                    # very last half is split into column-quarters so the
                    # final DMA starts while the last quarter computes.
                    for ht in range(2):
                        qs = 2 if (ht == 1 and d == 1) else 1
                        qw = COLS // qs
                        for qi in range(qs):
                            lo = ht * COLS + qi * qw
                            sl = slice(lo, lo + qw)
                            sq = slice(qi * qw, (qi + 1) * qw)
                            nc.scalar.activation(so7[:, sl], ug[GO][:, sl], SIG)
                            nc.vector.tensor_mul(hf[:, sl], so7[:, sl],
                                                 tchs[ht][:, sq])
                            nc.sync.dma_start(out=out[d, :, sl], in_=hf[:, sl])
                    h_fin32[d] = hf
                    continue

                # step-6 fwd activations stay fp32 (they feed the out6 fix)
                adt = f32 if t == SEG_LEN - 2 else bf16
                si = acts.tile([128, F], bf16, name=f"si_{t}_{d}", tag="si")
                nc.scalar.activation(si, ug[GI], SIG)
                # tanh(g) next: it gates the critical path to h via t2
                tg = acts.tile([128, F], bf16, name=f"tg_{t}_{d}", tag="tg")
                nc.scalar.activation(tg, ug[GG], TANH)
                if t > 0:
                    sf = acts.tile([128, F], bf16, name=f"sf_{t}_{d}", tag="sf")
                    nc.scalar.activation(sf, ug[GF], SIG)
                so = acts.tile([128, F], adt, name=f"so_{t}_{d}",
                               tag="so32" if adt is f32 else "so", bufs=4)
                nc.scalar.activation(so, ug[GO], SIG)

                # cell update: c (fp32) = sig_f * c + sig_i * tanh_g
                t2 = work.tile([128, F], bf16, name=f"t2_{t}_{d}", tag="t2",
                               bufs=4)
                nc.vector.tensor_mul(t2, si, tg)
                if t == 0:
                    c_new = t2
                else:
                    t1 = work.tile([128, F], f32, name=f"t1_{t}_{d}", tag="t1")
                    nc.vector.tensor_mul(t1, sf, c_prev[d])
                    c_new = work.tile([128, F], f32, name=f"c_{t}_{d}", tag=f"c{d}")
                    nc.vector.tensor_add(c_new, t1, t2)

                tc_ = acts.tile([128, F], adt, name=f"tc_{t}_{d}",
                                tag="tc32" if adt is f32 else "tc", bufs=4)
                h_bf = work.tile([128, F], bf16, name=f"h_{t}_{d}", tag=f"h{d}")
                if t == 0:
                    # split per hid-half: the t=1 hh matmul on k-tile 0 only
                    # needs h[:, 0:COLS], so let it start one op earlier
                    for ht in range(2):
                        sl = slice(ht * COLS, (ht + 1) * COLS)
                        nc.scalar.activation(tc_[:, sl], c_new[:, sl], TANH)
                        nc.vector.tensor_mul(h_bf[:, sl], so[:, sl], tc_[:, sl])
                else:
                    nc.scalar.activation(tc_, c_new, TANH)
                    nc.vector.tensor_mul(h_bf, so, tc_)

                if d == 1 and t == 0:
                    # bwd step 0 is masked for (core 0) column 0: zero the
                    # col-0 slice of h, c in place (per-core mask data)
                    c0 = c_new[:, 0:F:COLS]
                    nc.vector.tensor_mul(c0, c0, msk_sb)
                    h0 = h_bf[:, 0:F:COLS]
                    nc.vector.tensor_mul(h0, h0, Kbf)

                if d == 0 and t == SEG_LEN - 2:
                    # fwd h after step 6, col 0 only (ragged col-0, host fix)
                    h6c = work.tile([128, 2], f32, name="h6c", tag="h6c")
                    nc.vector.tensor_mul(h6c, so[:, 0:F:COLS], tc_[:, 0:F:COLS])
                    nc.gpsimd.dma_start(out=out6, in_=h6c)

                c_prev[d] = c_new
                h_prev[d] = h_bf

        # (outputs are DMA'd inside the loop; the host applies the core-0
        # column-0 ragged fix from out6)


def _prep_weights(inputs):
    """Per-vocab input-gates table (bf16) + gate-permuted recurrent weights.
    Cached on the identity of the input arrays."""
    key = tuple(id(inputs[k]) for k in
                ("emb_table", "w_ih_f", "w_hh_f", "b_ih_f", "b_hh_f",
                 "w_ih_b", "w_hh_b", "b_ih_b", "b_hh_b"))
    hit = _CACHE.get("weights")
    if hit is not None and hit[0] == key:
        return hit[2]
    # keep strong refs to the inputs so ids cannot be recycled
    refs = [inputs[k] for k in
            ("emb_table", "w_ih_f", "w_hh_f", "b_ih_f", "b_hh_f",
             "w_ih_b", "w_hh_b", "b_ih_b", "b_hh_b")]

    perm = np.concatenate([np.arange(0, 2 * HID),            # i, f
                           np.arange(3 * HID, 4 * HID),      # o
                           np.arange(2 * HID, 3 * HID)])     # g

    def aug(w_ih, b_ih, b_hh):
        a = np.empty((G4, EMBED + 1), dtype=np.float32)
        a[:, :EMBED] = np.asarray(w_ih, np.float32)
        a[:, EMBED] = np.asarray(b_ih, np.float32) + np.asarray(b_hh, np.float32)
        return a[perm]

    AUG = np.concatenate(
        [aug(inputs["w_ih_f"], inputs["b_ih_f"], inputs["b_hh_f"]),
         aug(inputs["w_ih_b"], inputs["b_ih_b"], inputs["b_hh_b"])])  # [G8, 301]
    E1 = np.empty((VOCAB, EMBED + 1), dtype=np.float32)
    E1[:, :EMBED] = np.asarray(inputs["emb_table"], np.float32)
    E1[:, EMBED] = 1.0
    # the per-vocab gates table: one exact fp32 GEMM, rounded once to bf16
    GT = (E1 @ AUG.T).astype(BF16)                            # [VOCAB, G8]

    def prep_whh(w_hh):
        a = np.asarray(w_hh, np.float32)[perm].T.reshape(KT_HH, 128, G4)
        return np.ascontiguousarray(
            a.transpose(1, 0, 2).reshape(128, KT_HH * G4)).astype(BF16)

    whh_arr = np.stack([prep_whh(inputs["w_hh_f"]), prep_whh(inputs["w_hh_b"])])

    # keep mask for column 0 (both hid tiles): 0 on core 0, 1 elsewhere
    msk_plain = np.ones((128, 2), dtype=np.float32)
    msk_core0 = np.zeros((128, 2), dtype=np.float32)

    prepped = (GT, whh_arr, msk_core0, msk_plain)
    _CACHE["weights"] = (key, refs, prepped)
    return prepped


def _tokens_per_core(seq):
    """v[c][l, n] = token id for (core c, step l, column n)."""
    vs = []
    for c in range(NCORES):
        if c == 0:
            w = np.concatenate([seq[0:1], seq[0:TOK - 1]])
        else:
            w = seq[TOK * c - 1: TOK * c + TOK - 1]
        v = w.reshape(COLS, SEG_LEN).T.copy()   # v[l, n]
        if c == 0:
            v[:, 0] = seq[0:SEG_LEN]            # col 0: seq[0..7], step 7 masked
        vs.append(v)
    return np.stack(vs)                          # [NCORES, SEG_LEN, COLS]


def _prep_host(inputs):
    """Build the per-core input maps from the full problem inputs."""
    GT, whh_arr, msk_core0, msk_plain = _prep_weights(inputs)
    seq = np.asarray(inputs["seq_s"]).astype(np.int64)

    V = _tokens_per_core(seq)                    # [8, L, N]
    TG = GT[V]                                   # [8, L, N, G8] bf16
    # xg[c][l, d, p, m*COLS + n] = TG[c, l, n, d*1024 + m*128 + p]
    X = np.ascontiguousarray(
        TG.reshape(NCORES, SEG_LEN, COLS, 2, 8, 128)
          .transpose(0, 1, 3, 5, 4, 2)
    ).reshape(NCORES, SEG_LEN, 2, 128, 8 * COLS)

    in_maps = []
    for c in range(NCORES):
        in_maps.append({
            "xg": X[c],
            "whh": whh_arr,
            "msk": msk_core0 if c == 0 else msk_plain,
        })
    return in_maps


def kernel(**inputs) -> np.ndarray:
    from concourse import bass_utils

    if "nc" not in _CACHE:
        _CACHE["nc"] = _build_program()
    nc = _CACHE["nc"]

    in_maps = _prep_host(inputs)
    res = bass_utils.run_bass_kernel_spmd(nc, in_maps, core_ids=list(range(NCORES)))
    return np.concatenate(
        [_untangle_out(r["out"], r["out6"] if c == 0 else None)
         for c, r in enumerate(res.results)], axis=0)


def _untangle_out(o, o6=None):
    """Device out [2, 128, F] -> [COLS, 2*HID]:
    o[d, p, ht*COLS + n] = h[d][hid = ht*128 + p, col n].
    o6 (core 0 only): [128, ht] = fwd step-6 hidden of column 0, which
    replaces the fwd features of the ragged column 0."""
    o4 = np.asarray(o).reshape(2, 128, 2, COLS)
    r = np.ascontiguousarray(o4.transpose(3, 0, 2, 1)).reshape(COLS, 2 * HID)
    if o6 is not None:
        r[0, :HID] = np.asarray(o6).T.reshape(HID)
    return r


if __name__ == "__main__":
    nc = _build_program()
    print("program built ok")
